# revision 39
# baseline (speedup 1.0000x reference)
"""Trainium2 Bass kernel for nn_Attn_time (sparse time-similarity attention).

reference:
    energies[i, j] = time_sim_mat[cur[i], his[j]]   # [4096, 8192]
    out = softmax(energies, axis=-1)

Structure exploited: cur/his index into only T=1024 time buckets, so
    out[i, j] = S[cur[i], j]  where  S = softmax_rows(time_sim_mat[:, his])
and S is only [1024, 8192]. Column-shard S across the 8 cores (1024 j each).

Per core (j-shard of 1024, processed as 2 pipelined halves of 512):
 - G[t, j] = sum_u M[t, u] * H[u, j] with one-hot H[u, j] = (his[j] == u),
   on the TensorEngine in fp16 (one-hot selection exact; fp16 rounds
   energies to ~5e-4 relative). M^T is uploaded pre-transposed and
   loaded as two DMAs on separate queues; his is uploaded
   pre-broadcast so the one-hot build starts early.
 - Softmax denominator rowsum[t] = sum_u exp(M[t,u])*cnt[u] is computed
   on the host; -ln(rowsum) enters as the per-partition ACT bias:
   S = exp(G - ln rowsum) in one activation op.
 - S-half rows (bf16) park in raw DRAM scratch (untracked by Tile) and
   are row-gathered by `cur` with dma_gather. SWDGE desc-gen is ~9us
   per 1024-idx prep (HW max), serial per queue, parallel across
   queues 1-3 (queue 0 crashes the runtime), and starts only ~22us in
   (Q7 library load). Half 0 owns the queue 1+2 pair (2-chains, done
   ~40us) so its gather+store overlaps the PE phase of half 1; half 1
   owns queue 3 (4-chain). count=None triggers; the first trigger of
   each half carries sync-deps on that half's parks.
 - Output is bf16 (the data is bf16-rounded anyway from the S park);
   the host widens to f32. Half 0 stores on the Pool DMA ring, half 1
   on the Sync ring, so the two halves' stores drain in parallel.
Per-core output shard: out[:, k*1024:(k+1)*1024]; host concatenates.
"""

import numpy as np

import concourse.bass as bass
import concourse.tile as tile
from concourse import bacc, mybir
from concourse.bass_utils import run_bass_kernel_spmd
from bass_rust import add_dep_helper

T = 1024          # time buckets
SEQ = 8192        # len(his)
STATE = 4096      # len(cur)
NCORES = 8
JSH = SEQ // NCORES        # j columns per core = 1024
NH = 2                     # pipelined j-halves per core
JH = JSH // NH             # 512
NCH = 4                    # gather chunks per half (1024 idxs each — HW max)
QMAP = {(0, 0): 1, (0, 1): 2, (0, 2): 1, (0, 3): 2,
        (1, 0): 3, (1, 1): 3, (1, 2): 3, (1, 3): 3}
TRIGS = {0: [1, 2], 1: [3]}         # queues to fire per half (count=None)

F32 = mybir.dt.float32
F16 = mybir.dt.float16
BF16 = mybir.dt.bfloat16
I16 = mybir.dt.int16


def build_kernel():
    nc = bacc.Bacc("TRN2", target_bir_lowering=False, debug=False,
                   num_devices=NCORES, num_swdge_queues=4,
                   dynamic_dma_scratch_size=32768)

    mt_param = nc.dram_tensor("mt16", [128, 8 * T], F16, kind="ExternalInput")
    his_param = nc.dram_tensor("hisb16", [128, JSH], F16,
                               kind="ExternalInput")
    cur_param = nc.dram_tensor("cur_idx16", [128, STATE // 16], I16,
                               kind="ExternalInput")
    ucol_param = nc.dram_tensor("ucol32", [128, 8], F32, kind="ExternalInput")
    lnrs_param = nc.dram_tensor("neg_lnrs", [128, 8], F32,
                                kind="ExternalInput")
    out_param = nc.dram_tensor("out", [STATE, JSH], BF16,
                               kind="ExternalOutput")
    # raw (Tile-untracked) DRAM scratch for the parked S halves; ordering
    # against the gathers is enforced explicitly via the trigger deps
    s_dram = [nc.dram_tensor(f"sdram{h}", [T, JH], BF16, kind="Internal")
              for h in range(NH)]

    with tile.TileContext(nc, num_cores=NCORES) as tc:
        with (
            tc.tile_pool(name="singles", bufs=1) as singles,
            tc.tile_pool(name="gat", bufs=1) as gat,
            tc.tile_pool(name="psum", bufs=4, space="PSUM") as psum,
        ):
            # ---- persistent SBUF tiles (h/eg split per-index so Tile's
            # range tracking can't invent cross-block dependencies)
            mt_sb = singles.tile([128, 8, T], F16)      # M^T [u%128, u//128, t]
            his_sb = singles.tile([128, JSH], F16)      # his bcast to all parts
            h_sb = [[singles.tile([128, JH], F16, name=f"h{h}_{c}",
                                  tag=f"h{h}_{c}") for c in range(8)]
                    for h in range(NH)]
            eg_sb = [[singles.tile([128, JH], BF16, name=f"eg{h}_{m}",
                                   tag=f"eg{h}_{m}") for m in range(8)]
                     for h in range(NH)]
            idx_sb = singles.tile([128, STATE // 16], I16)
            ucol_sb = singles.tile([128, 8], F32)       # ucol[p,c] = c*128+p
            lnrs_sb = singles.tile([128, 8], F32)       # -ln rowsum, t=m*128+p

            # ---- loads: idx first on sync (gates SWDGE desc-gen); his
            # first on scalar (gates the one-hot build); M^T split across
            # both queues
            nc.sync.dma_start(out=idx_sb, in_=cur_param.ap())
            nc.sync.dma_start(out=mt_sb[:, 0:5, :],
                              in_=mt_param.ap()[:, 0:5 * T])
            nc.scalar.dma_start(out=his_sb, in_=his_param.ap())
            nc.scalar.dma_start(out=mt_sb[:, 5:8, :],
                                in_=mt_param.ap()[:, 5 * T:8 * T])
            nc.scalar.dma_start(out=ucol_sb, in_=ucol_param.ap())
            nc.scalar.dma_start(out=lnrs_sb, in_=lnrs_param.ap())

            gat_sems = {(h, ch): nc.alloc_semaphore(f"gat{h}_{ch}")
                        for h in range(NH) for ch in range(NCH)}

            # ---- all 8 gather preps launch up front; Q7 desc-gen overlaps
            # the PE phase
            gtiles = {}
            for h in (0, 1):
                for ch in range(NCH):
                    gst = ch * 1024
                    g = gat.tile([128, 8, JH], BF16,
                                 name=f"g{h}_{ch}", tag=f"g{h}_{ch}")
                    nc.gpsimd.dma_gather(
                        g,
                        s_dram[h].ap(),
                        idx_sb[:, gst // 16:gst // 16 + 64],
                        num_idxs=1024,
                        num_idxs_reg=1024,
                        elem_size=JH,
                        elem_step=JH,
                        prepare_only=True,
                        sem=gat_sems[h, ch],
                        queue_num=QMAP[h, ch],
                    )
                    gtiles[h, ch] = g

            # ---- one-hot H[u, j] = (his[j] == u), u = c*128+p, per half
            for h in range(NH):
                for c in range(8):
                    nc.vector.tensor_scalar(
                        out=h_sb[h][c],
                        in0=his_sb[:, h * JH:(h + 1) * JH],
                        scalar1=ucol_sb[:, c:c + 1],
                        scalar2=None,
                        op0=mybir.AluOpType.is_equal,
                    )

            # ---- G = M @ H on PE (fp16, f32 accum); S = exp(G - ln rowsum)
            # via the ACT bias; park each t-block as it finishes
            park_insts = {h: [] for h in range(NH)}
            for h in range(NH):
                for m in range(8):
                    pg = psum.tile([128, JH], F32)
                    for c in range(8):
                        nc.tensor.matmul(
                            pg,
                            mt_sb[:, c, m * 128:(m + 1) * 128],
                            h_sb[h][c],
                            start=(c == 0),
                            stop=(c == 7),
                        )
                    nc.scalar.activation(
                        out=eg_sb[h][m],
                        in_=pg,
                        func=mybir.ActivationFunctionType.Exp,
                        bias=lnrs_sb[:, m:m + 1],
                    )
                    pk = nc.sync.dma_start(
                        out=s_dram[h].ap()[m * 128:(m + 1) * 128, :],
                        in_=eg_sb[h][m],
                    )
                    park_insts[h].append(pk)

            # ---- ALL triggers on the Pool stream (nothing else queues
            # there, so trig1 fires at parks1/desc-gen, not behind store
            # waits). The first trigger of each half carries sync-deps on
            # that half's parks.
            trig_of = {}
            prev = None
            for h in range(NH):
                for ti, q in enumerate(TRIGS[h]):
                    trig = nc.gpsimd.trigger_dma(count=None, queue_num=q)
                    if ti == 0:
                        for pk in park_insts[h]:
                            add_dep_helper(trig.ins, pk.ins, True,
                                           "fire gathers only after S landed")
                    if prev is not None:
                        add_dep_helper(trig.ins, prev.ins, False,
                                       "triggers run in order")
                    prev = trig
                trig_of[h] = trig

            # ---- store bf16 -> bf16 after each gather chunk lands; half 0
            # drains on the Scalar ring (free after the ACTs), half 1 on
            # the Sync ring (free after the parks)
            for h in range(NH):
                eng = nc.scalar if h == 0 else nc.sync
                prev_dep = trig_of[h]
                for ch in range(NCH):
                    gst = ch * 1024
                    ws = eng.wait_ge(gat_sems[h, ch], 16)
                    add_dep_helper(ws.ins, prev_dep.ins, False,
                                   "wait only makes progress once fired")
                    out_view = out_param.ap()[gst:gst + 1024,
                                              h * JH:(h + 1) * JH]
                    st = eng.dma_start(
                        out=out_view.rearrange("(q p) j -> p q j", p=128),
                        in_=gtiles[h, ch],
                    )
                    add_dep_helper(st.ins, ws.ins, False,
                                   "store only after its gather chunk landed")
                    prev_dep = st

    nc.compile()
    return nc


_NC_CACHE = None
_last_in_maps = None


def _get_nc():
    global _NC_CACHE
    if _NC_CACHE is None:
        _NC_CACHE = build_kernel()
    return _NC_CACHE


def kernel(his, cur, time_sim_mat):
    his = np.asarray(his)
    cur = np.asarray(cur)
    m = np.asarray(time_sim_mat, dtype=np.float32)

    # M^T in fp16, laid out [p, c, t] with u = c*128+p contraction index
    mt = m.T.astype(np.float16)                       # mt[u, t]
    mt16 = np.ascontiguousarray(
        mt.reshape(8, 128, T).transpose(1, 0, 2)).reshape(128, 8 * T)

    # cur indices, wrapped for dma_gather: chunk ch uses idx columns
    # [ch*64, (ch+1)*64); index g of a chunk sits at [g%16, g//16].
    a = cur.astype(np.int16).reshape(STATE // 16, 16).T
    cur16 = np.tile(np.ascontiguousarray(a), (8, 1))  # replicate to 8 groups

    p = np.arange(128, dtype=np.float32)
    ucol32 = np.ascontiguousarray(
        p[:, None] + 128.0 * np.arange(8, dtype=np.float32)[None, :])

    # softmax denominator on the host: rowsum[t] = sum_u exp(M[t,u]) * cnt[u]
    cnt = np.bincount(np.asarray(his, dtype=np.int64), minlength=T)
    rowsum = (np.exp(m.astype(np.float64)) @ cnt.astype(np.float64))
    neg_lnrs = (-np.log(rowsum)).astype(np.float32)
    lnrs_col = np.ascontiguousarray(neg_lnrs.reshape(8, 128).T)

    in_maps = []
    for k in range(NCORES):
        hisb = np.broadcast_to(
            his[k * JSH:(k + 1) * JSH].astype(np.float16)[None, :],
            (128, JSH))
        in_maps.append({
            "mt16": mt16,
            "hisb16": np.ascontiguousarray(hisb),
            "cur_idx16": cur16,
            "ucol32": ucol32,
            "neg_lnrs": lnrs_col,
        })

    global _last_in_maps
    _last_in_maps = in_maps

    nc = _get_nc()
    res = run_bass_kernel_spmd(nc, in_maps, core_ids=list(range(NCORES)))
    out = np.concatenate(
        [np.asarray(res.results[k]["out"]).astype(np.float32)
         for k in range(NCORES)], axis=1)
    return out


# revision 40
# speedup vs baseline: 1.1139x; 1.1139x over previous
"""Trainium2 Bass kernel for nn_Attn_time (sparse time-similarity attention).

reference:
    energies[i, j] = time_sim_mat[cur[i], his[j]]   # [4096, 8192]
    out = softmax(energies, axis=-1)

Structure exploited: cur/his index into only T=1024 time buckets, so
    out[i, j] = S[cur[i], j]  where  S = softmax_rows(time_sim_mat[:, his])
and S is only [1024, 8192]. Column-shard S across the 8 cores (1024 j each).

Per core (j-shard of 1024, processed as 2 pipelined halves of 512):
 - G[t, j] = sum_u M[t, u] * H[u, j] with one-hot H[u, j] = (his[j] == u),
   on the TensorEngine in fp16 (one-hot selection exact; fp16 rounds
   energies to ~5e-4 relative). M^T is uploaded pre-transposed and
   loaded as two DMAs on separate queues; his is uploaded
   pre-broadcast so the one-hot build starts early.
 - Softmax denominator rowsum[t] = sum_u exp(M[t,u])*cnt[u] is computed
   on the host; -ln(rowsum) enters as the per-partition ACT bias:
   S = exp(G - ln rowsum) in one activation op.
 - S-half rows (bf16) park in raw DRAM scratch (untracked by Tile) and
   are row-gathered by `cur` with dma_gather. SWDGE desc-gen is ~9us
   per 1024-idx prep (HW max), serial per queue, parallel across
   queues 1-3 (queue 0 crashes the runtime), and starts only ~22us in
   (Q7 library load). Half 0 owns the queue 1+2 pair (2-chains, done
   ~40us) so its gather+store overlaps the PE phase of half 1; half 1
   owns queue 3 (4-chain). count=None triggers; the first trigger of
   each half carries sync-deps on that half's parks.
 - Output is bf16 (the data is bf16-rounded anyway from the S park);
   the host widens to f32. Half 0 stores on the Pool DMA ring, half 1
   on the Sync ring, so the two halves' stores drain in parallel.
Per-core output shard: out[:, k*1024:(k+1)*1024]; host concatenates.
"""

import numpy as np

import concourse.bass as bass
import concourse.tile as tile
from concourse import bacc, mybir
from concourse.bass_utils import run_bass_kernel_spmd
from bass_rust import add_dep_helper

T = 1024          # time buckets
SEQ = 8192        # len(his)
STATE = 4096      # len(cur)
NCORES = 8
JSH = SEQ // NCORES        # j columns per core = 1024
NH = 2                     # pipelined j-halves per core
JH = JSH // NH             # 512
NCH = 4                    # gather chunks per half (1024 idxs each — HW max)
QMAP = {(0, 0): 1, (0, 1): 2, (0, 2): 1, (0, 3): 2,
        (1, 0): 3, (1, 1): 3, (1, 2): 3, (1, 3): 3}
TRIGS = {0: [1, 2], 1: [3]}         # queues to fire per half (count=None)

F32 = mybir.dt.float32
F16 = mybir.dt.float16
BF16 = mybir.dt.bfloat16
I16 = mybir.dt.int16


def build_kernel():
    nc = bacc.Bacc("TRN2", target_bir_lowering=False, debug=False,
                   num_devices=NCORES, num_swdge_queues=4,
                   dynamic_dma_scratch_size=32768)

    mt_param = nc.dram_tensor("mt16", [128, 8 * T], F16, kind="ExternalInput")
    his_param = nc.dram_tensor("hisb16", [128, JSH], F16,
                               kind="ExternalInput")
    cur_param = nc.dram_tensor("cur_idx16", [128, STATE // 16], I16,
                               kind="ExternalInput")
    ucol_param = nc.dram_tensor("ucol32", [128, 8], F32, kind="ExternalInput")
    lnrs_param = nc.dram_tensor("neg_lnrs", [128, 8], F32,
                                kind="ExternalInput")
    out_param = nc.dram_tensor("out", [STATE, JSH], BF16,
                               kind="ExternalOutput")
    # raw (Tile-untracked) DRAM scratch for the parked S halves; ordering
    # against the gathers is enforced explicitly via the trigger deps
    s_dram = [nc.dram_tensor(f"sdram{h}", [T, JH], BF16, kind="Internal")
              for h in range(NH)]

    with tile.TileContext(nc, num_cores=NCORES) as tc:
        with (
            tc.tile_pool(name="singles", bufs=1) as singles,
            tc.tile_pool(name="gat", bufs=1) as gat,
            tc.tile_pool(name="psum", bufs=4, space="PSUM") as psum,
        ):
            # ---- persistent SBUF tiles (h/eg split per-index so Tile's
            # range tracking can't invent cross-block dependencies)
            mt_sb = singles.tile([128, 8, T], F16)      # M^T [u%128, u//128, t]
            his_sb = singles.tile([128, JSH], F16)      # his bcast to all parts
            h_sb = [[singles.tile([128, JH], F16, name=f"h{h}_{c}",
                                  tag=f"h{h}_{c}") for c in range(8)]
                    for h in range(NH)]
            eg_sb = [[singles.tile([128, JH], BF16, name=f"eg{h}_{m}",
                                   tag=f"eg{h}_{m}") for m in range(8)]
                     for h in range(NH)]
            idx_sb = singles.tile([128, STATE // 16], I16)
            ucol_sb = singles.tile([128, 8], F32)       # ucol[p,c] = c*128+p
            lnrs_sb = singles.tile([128, 8], F32)       # -ln rowsum, t=m*128+p

            # ---- loads: idx first on sync (gates SWDGE desc-gen); his
            # first on scalar (gates the one-hot build); M^T split across
            # both queues
            nc.sync.dma_start(out=idx_sb, in_=cur_param.ap())
            nc.sync.dma_start(out=mt_sb[:, 0:4, :],
                              in_=mt_param.ap()[:, 0:4 * T])
            nc.scalar.dma_start(out=his_sb, in_=his_param.ap())
            nc.scalar.dma_start(out=mt_sb[:, 4:8, :],
                                in_=mt_param.ap()[:, 4 * T:8 * T])
            nc.scalar.dma_start(out=ucol_sb, in_=ucol_param.ap())
            nc.scalar.dma_start(out=lnrs_sb, in_=lnrs_param.ap())

            gat_sems = {(h, ch): nc.alloc_semaphore(f"gat{h}_{ch}")
                        for h in range(NH) for ch in range(NCH)}

            # ---- all 8 gather preps launch up front; Q7 desc-gen overlaps
            # the PE phase
            gtiles = {}
            for h in (0, 1):
                for ch in range(NCH):
                    gst = ch * 1024
                    g = gat.tile([128, 8, JH], BF16,
                                 name=f"g{h}_{ch}", tag=f"g{h}_{ch}")
                    nc.gpsimd.dma_gather(
                        g,
                        s_dram[h].ap(),
                        idx_sb[:, gst // 16:gst // 16 + 64],
                        num_idxs=1024,
                        num_idxs_reg=1024,
                        elem_size=JH,
                        elem_step=JH,
                        prepare_only=True,
                        sem=gat_sems[h, ch],
                        queue_num=QMAP[h, ch],
                    )
                    gtiles[h, ch] = g

            # ---- one-hot H[u, j] = (his[j] == u), u = c*128+p, per half
            for h in range(NH):
                for c in range(8):
                    nc.vector.tensor_scalar(
                        out=h_sb[h][c],
                        in0=his_sb[:, h * JH:(h + 1) * JH],
                        scalar1=ucol_sb[:, c:c + 1],
                        scalar2=None,
                        op0=mybir.AluOpType.is_equal,
                    )

            # ---- G = M @ H on PE (fp16, f32 accum); S = exp(G - ln rowsum)
            # via the ACT bias; park each t-block as it finishes
            park_insts = {h: [] for h in range(NH)}
            for h in range(NH):
                for m in range(8):
                    pg = psum.tile([128, JH], F32)
                    for c in range(8):
                        nc.tensor.matmul(
                            pg,
                            mt_sb[:, c, m * 128:(m + 1) * 128],
                            h_sb[h][c],
                            start=(c == 0),
                            stop=(c == 7),
                        )
                    nc.scalar.activation(
                        out=eg_sb[h][m],
                        in_=pg,
                        func=mybir.ActivationFunctionType.Exp,
                        bias=lnrs_sb[:, m:m + 1],
                    )
                    pk = nc.sync.dma_start(
                        out=s_dram[h].ap()[m * 128:(m + 1) * 128, :],
                        in_=eg_sb[h][m],
                    )
                    park_insts[h].append(pk)

            # ---- per half: fire the gathers once its parks completed
            # (parks complete in ring order, so a sync-dep on the LAST park
            # implies all eight and minimizes sem aliasing), then its
            # stores (half 0 on the Pool ring, half 1 on the Sync ring)
            prev = None
            for h in range(NH):
                for ti, q in enumerate(TRIGS[h]):
                    trig = nc.gpsimd.trigger_dma(count=None, queue_num=q)
                    if ti == 0:
                        add_dep_helper(trig.ins, park_insts[h][-1].ins, True,
                                       "fire gathers only after S landed")
                    if prev is not None:
                        add_dep_helper(trig.ins, prev.ins, False,
                                       "triggers run in order")
                    prev = trig

                eng = nc.gpsimd if h == 0 else nc.sync
                prev_dep = prev
                for ch in range(NCH):
                    gst = ch * 1024
                    ws = eng.wait_ge(gat_sems[h, ch], 16)
                    add_dep_helper(ws.ins, prev_dep.ins, False,
                                   "wait only makes progress once fired")
                    out_view = out_param.ap()[gst:gst + 1024,
                                              h * JH:(h + 1) * JH]
                    st = eng.dma_start(
                        out=out_view.rearrange("(q p) j -> p q j", p=128),
                        in_=gtiles[h, ch],
                    )
                    add_dep_helper(st.ins, ws.ins, False,
                                   "store only after its gather chunk landed")
                    prev_dep = st

    nc.compile()
    return nc


_NC_CACHE = None
_last_in_maps = None


def _get_nc():
    global _NC_CACHE
    if _NC_CACHE is None:
        _NC_CACHE = build_kernel()
    return _NC_CACHE


def kernel(his, cur, time_sim_mat):
    his = np.asarray(his)
    cur = np.asarray(cur)
    m = np.asarray(time_sim_mat, dtype=np.float32)

    # M^T in fp16, laid out [p, c, t] with u = c*128+p contraction index
    mt = m.T.astype(np.float16)                       # mt[u, t]
    mt16 = np.ascontiguousarray(
        mt.reshape(8, 128, T).transpose(1, 0, 2)).reshape(128, 8 * T)

    # cur indices, wrapped for dma_gather: chunk ch uses idx columns
    # [ch*64, (ch+1)*64); index g of a chunk sits at [g%16, g//16].
    a = cur.astype(np.int16).reshape(STATE // 16, 16).T
    cur16 = np.tile(np.ascontiguousarray(a), (8, 1))  # replicate to 8 groups

    p = np.arange(128, dtype=np.float32)
    ucol32 = np.ascontiguousarray(
        p[:, None] + 128.0 * np.arange(8, dtype=np.float32)[None, :])

    # softmax denominator on the host: rowsum[t] = sum_u exp(M[t,u]) * cnt[u]
    cnt = np.bincount(np.asarray(his, dtype=np.int64), minlength=T)
    rowsum = (np.exp(m.astype(np.float64)) @ cnt.astype(np.float64))
    neg_lnrs = (-np.log(rowsum)).astype(np.float32)
    lnrs_col = np.ascontiguousarray(neg_lnrs.reshape(8, 128).T)

    in_maps = []
    for k in range(NCORES):
        hisb = np.broadcast_to(
            his[k * JSH:(k + 1) * JSH].astype(np.float16)[None, :],
            (128, JSH))
        in_maps.append({
            "mt16": mt16,
            "hisb16": np.ascontiguousarray(hisb),
            "cur_idx16": cur16,
            "ucol32": ucol32,
            "neg_lnrs": lnrs_col,
        })

    global _last_in_maps
    _last_in_maps = in_maps

    nc = _get_nc()
    res = run_bass_kernel_spmd(nc, in_maps, core_ids=list(range(NCORES)))
    out = np.concatenate(
        [np.asarray(res.results[k]["out"]).astype(np.float32)
         for k in range(NCORES)], axis=1)
    return out
